# revision 3
# baseline (speedup 1.0000x reference)
"""Causal self-attention (B=4, T=2048, C=1024, H=16) on 8 NeuronCores.

Sharding: core c handles batch b = c//2 and head-half half = c%2 (8 heads,
512 channels). QKV projections are column-parallel, output projection is
row-parallel (Megatron); the two per-batch output partials are summed on host.

Fast path (all-zero biases, which is what the reference's setup_inputs
produces): a single fused pipeline per iteration, all activations/weights in
bf16 (PSUM accumulation stays f32), emitted so the PE never drains:

  stage A: q/k/v projections for t in [0, 1024)
  stage B: attention queries [0, 1024), interleaved with projections for
           t in [1024, 2048) as PE filler work
  stage C: attention queries [1024, 2048), interleaved with the output
           projection for t in [0, 1024)
  stage D: output projection for t in [1024, 2048)

Attention per (chunk, head): per 128-key tile j, S^T = kT^T q into PSUM with
the causal mask of the diagonal block added in-PSUM by one extra matmul
(strict-upper -1e9 stationary x identity moving), exp via the Act engine
(fixed softmax max of 0; scores are ~N(0,1)), y accumulated via vAug^T @ P
where vAug carries a ones-column so row 64 of the y PSUM is the softmax
denominator. Normalize on DVE/Pool, y stays in SBUF for the output projection.

Nonzero biases fall back to the original (slower) general build.
"""

import sys
import types

import numpy as np
from collections import deque
from contextlib import ExitStack, nullcontext

import concourse.bass as bass
import concourse.mybir as mybir
import concourse.tile as tile
from concourse import bacc
from concourse.bass_utils import run_bass_kernel_spmd

# If the environment sets BASS_TRACE but ships only the antenv stub (no
# axon_hooks), run_bass_kernel_spmd would crash on import. Provide the
# graceful "no hook registered" fallback only when the real module is absent.
try:  # pragma: no cover
    import antenv.axon_hooks  # noqa: F401
except ImportError:  # pragma: no cover
    import antenv

    _stub = types.ModuleType("antenv.axon_hooks")
    _stub.get_axon_ntff_profile_hook = lambda: None
    sys.modules["antenv.axon_hooks"] = _stub
    antenv.axon_hooks = _stub

F32 = mybir.dt.float32
F32R = mybir.dt.float32r
BF16 = mybir.dt.bfloat16
EXP = mybir.ActivationFunctionType.Exp

B, T, C, H = 4, 2048, 1024, 16
HD = C // H              # 64 head dim
N_CORES = 8
HPC = H // 2             # 8 heads per core
MPC = C // 2             # 512 channels per core
MT = MPC // 128          # 4 m-tiles per core
CT = C // 128            # 8 contraction tiles
TC = T // 512            # 4 t-chunks
TT = T // 128            # 16 t-tiles
SCALE = float(1.0 / np.sqrt(HD))

_CACHE = {}


def _build_fast(bench_loops=None):
    nc = bacc.Bacc()
    xT = nc.declare_dram_parameter("xT", [C, T], BF16, isOutput=False)
    wqT = nc.declare_dram_parameter("wqT", [C, MPC], BF16, isOutput=False)
    wkT = nc.declare_dram_parameter("wkT", [C, MPC], BF16, isOutput=False)
    wvT = nc.declare_dram_parameter("wvT", [C, MPC], BF16, isOutput=False)
    wpT = nc.declare_dram_parameter("wpT", [MPC, C], BF16, isOutput=False)
    maskT = nc.declare_dram_parameter("maskT", [128, 128], BF16, isOutput=False)
    iden = nc.declare_dram_parameter("iden", [128, 128], BF16, isOutput=False)
    outp = nc.declare_dram_parameter("out", [T, C], F32, isOutput=True)

    with tile.TileContext(nc) as tc:
        with ExitStack() as ctx:
            persist = ctx.enter_context(tc.tile_pool(name="persist", bufs=1))
            ps_S = ctx.enter_context(tc.tile_pool(name="ps_S", bufs=2, space="PSUM"))
            ps_y = ctx.enter_context(tc.tile_pool(name="ps_y", bufs=1, space="PSUM"))
            ps_fill = ctx.enter_context(tc.tile_pool(name="ps_fill", bufs=2, space="PSUM"))
            pool_xs = ctx.enter_context(tc.tile_pool(name="pool_xs", bufs=2))
            pool_P = ctx.enter_context(tc.tile_pool(name="pool_P", bufs=2))
            pool_tail = ctx.enter_context(tc.tile_pool(name="pool_tail", bufs=2))
            pool_o = ctx.enter_context(tc.tile_pool(name="pool_o", bufs=2))

            # ---- constants / weights (loaded once) ----
            mask_sb = persist.tile([128, 128], BF16, name="mask_sb")
            nc.sync.dma_start(out=mask_sb, in_=maskT[:, :])
            iden_sb = persist.tile([128, 128], BF16, name="iden_sb")
            nc.sync.dma_start(out=iden_sb, in_=iden[:, :])
            wq_t = [persist.tile([128, MPC], BF16, name=f"wq{c}") for c in range(CT)]
            wk_t = [persist.tile([128, MPC], BF16, name=f"wk{c}") for c in range(CT)]
            wv_t = [persist.tile([128, MPC], BF16, name=f"wv{c}") for c in range(CT)]
            for c in range(CT):
                nc.sync.dma_start(out=wq_t[c], in_=wqT[c * 128:(c + 1) * 128, :])
                nc.sync.dma_start(out=wk_t[c], in_=wkT[c * 128:(c + 1) * 128, :])
                nc.sync.dma_start(out=wv_t[c], in_=wvT[c * 128:(c + 1) * 128, :])
            wp_t = [persist.tile([128, C], BF16, name=f"wp{m}") for m in range(MT)]
            for m in range(MT):
                nc.sync.dma_start(out=wp_t[m], in_=wpT[m * 128:(m + 1) * 128, :])

            # ---- persistent activations ----
            qT_sb = [persist.tile([128, T], BF16, name=f"qT{m}") for m in range(MT)]
            kT_sb = [persist.tile([128, T], BF16, name=f"kT{m}") for m in range(MT)]
            vAug = [persist.tile([128, HPC * (HD + 1)], BF16, name=f"vAug{t}") for t in range(TT)]
            yst = [persist.tile([128, T], BF16, name=f"yst{m}") for m in range(MT)]
            # ones column of vAug never changes: write once, outside the loop
            for t_ in range(TT):
                va = vAug[t_].rearrange("p (h w) -> p h w", w=HD + 1)
                nc.vector.memset(va[:, :, HD], 1.0)

            rep = tc.For_i(0, bench_loops, 1) if bench_loops else nullcontext()
            with rep:
                # ---------- emission helpers ----------
                def emit_xs_dma(tch):
                    t0 = tch * 512
                    xs = []
                    for c in range(CT):
                        x_ = pool_xs.tile([128, 512], BF16, name=f"xs_{tch}_{c}",
                                          tag="xs", bufs=18)
                        nc.sync.dma_start(out=x_, in_=xT[c * 128:(c + 1) * 128, t0:t0 + 512])
                        xs.append(x_)
                    return xs

                def qk_group(tch, xs, wt, dst, m):
                    t0 = tch * 512
                    ps = ps_fill.tile([128, 512], F32, name=f"psqk_{tch}_{id(wt)}_{m}",
                                      tag="fill")
                    for c in range(CT):
                        nc.tensor.matmul(ps, wt[c][:, m * 128:(m + 1) * 128], xs[c],
                                         start=(c == 0), stop=(c == CT - 1))
                    nc.vector.tensor_copy(dst[m][:, t0:t0 + 512], ps)

                def v_group(tch, xs, tt_local):
                    tg = tch * 4 + tt_local
                    ps = ps_fill.tile([128, MPC], F32, name=f"psv_{tg}", tag="fill")
                    for c in range(CT):
                        nc.tensor.matmul(ps, xs[c][:, tt_local * 128:(tt_local + 1) * 128],
                                         wv_t[c], start=(c == 0), stop=(c == CT - 1))
                    va = vAug[tg].rearrange("p (h w) -> p h w", w=HD + 1)
                    nc.vector.tensor_copy(va[:, :, 0:HD],
                                          ps.rearrange("p (h w) -> p h w", w=HD))

                def p1_groups(tch, xs):
                    ops = []
                    for wt, dst in ((wq_t, qT_sb), (wk_t, kT_sb)):
                        for m in range(MT):
                            ops.append(lambda tch=tch, xs=xs, wt=wt, dst=dst, m=m:
                                       qk_group(tch, xs, wt, dst, m))
                    for tl in range(4):
                        ops.append(lambda tch=tch, xs=xs, tl=tl: v_group(tch, xs, tl))
                    return ops

                def p3_group(tt, nch):
                    n0 = nch * 512
                    ps = ps_fill.tile([128, 512], F32, name=f"pso_{tt}_{nch}", tag="fill")
                    for m in range(MT):
                        nc.tensor.matmul(ps, yst[m][:, tt * 128:(tt + 1) * 128],
                                         wp_t[m][:, n0:n0 + 512],
                                         start=(m == 0), stop=(m == MT - 1))
                    o_sb = pool_o.tile([128, 512], F32, name=f"o_{tt}_{nch}", tag="o", bufs=3)
                    nc.vector.tensor_copy(o_sb, ps)
                    nc.sync.dma_start(out=outp[tt * 128:(tt + 1) * 128, n0:n0 + 512], in_=o_sb)

                def attn(ic, fillq, n_slots, fill_start=0):
                    i0 = ic * 1024
                    jmax = 8 * ic + 7
                    n_fill0 = len(fillq)
                    slot = 0
                    done_fill = 0

                    def drain(slot):
                        nonlocal done_fill
                        if not fillq or slot < fill_start:
                            return
                        avail = max(1, n_slots - fill_start)
                        want = (min(slot - fill_start + 1, avail) * n_fill0 + avail - 1) // avail
                        while done_fill < want and fillq:
                            fillq.popleft()()
                            done_fill += 1

                    for h in range(HPC):
                        mt, so = h // 2, (h % 2) * 64
                        kT_h = kT_sb[mt]
                        qT_h = qT_sb[mt]
                        psy = ps_y.tile([HD + 1, 1024], F32, name=f"psy_{h}_{ic}", tag="psy")
                        pend = None

                        def emit_y(j, P):
                            lo = max(0, j * 128 - i0)
                            for a, b in ((lo, 512), (max(lo, 512), 1024)):
                                if a < b:
                                    nc.tensor.matmul(psy[:, a:b],
                                                     vAug[j][:, h * (HD + 1):(h + 1) * (HD + 1)],
                                                     P[:, a:b],
                                                     start=(j == 0), stop=(j == jmax))

                        for j in range(jmax + 1):
                            lo = max(0, j * 128 - i0)
                            ps = ps_S.tile([128, 1024], F32, name=f"pss_{h}_{ic}_{j}", tag="S")
                            if j * 128 >= i0:
                                # diagonal block: additive causal mask folded
                                # into the PSUM accumulation group
                                nc.tensor.matmul(ps[:, lo:lo + 128], mask_sb, iden_sb,
                                                 start=True, stop=False)
                                nc.tensor.matmul(ps[:, lo:lo + 128],
                                                 kT_h[so:so + HD, j * 128:(j + 1) * 128],
                                                 qT_h[so:so + HD, i0 + lo:i0 + lo + 128],
                                                 start=False, stop=True)
                                segs = ((lo + 128, 512), (max(lo + 128, 512), 1024))
                            else:
                                segs = ((lo, 512), (max(lo, 512), 1024))
                            for a, b in segs:
                                if a < b:
                                    nc.tensor.matmul(ps[:, a:b],
                                                     kT_h[so:so + HD, j * 128:(j + 1) * 128],
                                                     qT_h[so:so + HD, i0 + a:i0 + b],
                                                     start=True, stop=True)
                            P = pool_P.tile([128, 1024], BF16, name=f"P_{h}_{ic}_{j}",
                                            tag="P", bufs=3)
                            nc.scalar.activation(out=P[:, lo:1024], in_=ps[:, lo:1024],
                                                 func=EXP, scale=SCALE)
                            if pend is not None:
                                emit_y(*pend)
                            pend = (j, P)
                            drain(slot)
                            slot += 1
                        emit_y(*pend)
                        # normalize tail: decoupled from the accumulator ASAP
                        yu = pool_tail.tile([HD + 1, 1024], BF16, name=f"yu_{h}_{ic}",
                                            tag="yu", bufs=2)
                        nc.vector.tensor_copy(yu, psy)
                        r_ = pool_tail.tile([1, 1024], BF16, name=f"r_{h}_{ic}",
                                            tag="r", bufs=2)
                        with nc.allow_low_precision(reason="softmax denom reciprocal in bf16; rel err ~0.4% within budget"):
                            nc.vector.reciprocal(r_, yu[HD:HD + 1, :])
                        rb = pool_tail.tile([HD, 1024], BF16, name=f"rb_{h}_{ic}",
                                            tag="rb", bufs=2)
                        nc.gpsimd.partition_broadcast(rb, r_)
                        nc.vector.tensor_mul(yst[mt][so:so + HD, i0:i0 + 1024],
                                             yu[0:HD, :], rb)
                    while fillq:
                        fillq.popleft()()

                # ---------- schedule ----------
                # stage A: projections for t in [0, 1024)
                xs0 = emit_xs_dma(0)
                xs1 = emit_xs_dma(1)
                for op in p1_groups(0, xs0):
                    op()
                for op in p1_groups(1, xs1):
                    op()
                # stage B: attention ic=0, filled with projections t in [1024, 2048)
                xs2 = emit_xs_dma(2)
                xs3 = emit_xs_dma(3)
                fillq = deque(p1_groups(2, xs2) + p1_groups(3, xs3))
                attn(0, fillq, n_slots=64)
                # stage C: attention ic=1, filled with output projection t<1024
                fillq3 = deque(
                    (lambda tt=tt, nch=nch: p3_group(tt, nch))
                    for tt in range(8) for nch in range(2)
                )
                attn(1, fillq3, n_slots=128, fill_start=20)
                # stage D: output projection for t in [1024, 2048)
                for tt in range(8, 16):
                    for nch in range(2):
                        p3_group(tt, nch)
    nc.finalize()
    return nc


def make_in_maps(x, Wk, bk, Wq, bq, Wv, bv, Wp, bp):
    bf = mybir.dt.np(BF16)
    x = np.asarray(x, dtype=np.float32)
    Wk, Wq, Wv, Wp = (np.asarray(a, dtype=np.float32) for a in (Wk, Wq, Wv, Wp))

    # stationary mask for the diagonal block: lhsT[i, j] = -1e9 where j > i,
    # so (lhsT^T @ I)[key, q] = -1e9 where key > q
    mask = np.where(np.tril(np.ones((128, 128), dtype=bool)), 0.0, -1e9).astype(bf)
    iden = np.eye(128, dtype=np.float32).astype(bf)
    xT_b = [np.ascontiguousarray(x[b].T).astype(bf) for b in range(B)]
    in_maps = []
    for c in range(N_CORES):
        b, half = c // 2, c % 2
        hs = half * MPC
        in_maps.append({
            "xT": xT_b[b],
            "wqT": np.ascontiguousarray(Wq[hs:hs + MPC, :].T).astype(bf),
            "wkT": np.ascontiguousarray(Wk[hs:hs + MPC, :].T).astype(bf),
            "wvT": np.ascontiguousarray(Wv[hs:hs + MPC, :].T).astype(bf),
            "wpT": np.ascontiguousarray(Wp[:, hs:hs + MPC].T).astype(bf),
            "maskT": mask,
            "iden": iden,
        })
    return in_maps


def _get_nc(bench_loops=None, phases=(1, 2, 3)):
    if tuple(phases) == (1, 2, 3):
        key = ("fast", bench_loops)
        if key not in _CACHE:
            _CACHE[key] = _build_fast(bench_loops)
        return _CACHE[key]
    key = ("gen", bench_loops, tuple(phases))
    if key not in _CACHE:
        _CACHE[key] = _build_general(bench_loops, phases)
    return _CACHE[key]


def kernel(x, Wk, bk, Wq, bq, Wv, bv, Wp, bp, **run_kwargs):
    zero_bias = all(np.all(np.asarray(b_) == 0) for b_ in (bk, bq, bv, bp))
    if zero_bias:
        in_maps = make_in_maps(x, Wk, bk, Wq, bq, Wv, bv, Wp, bp)
        nc = _get_nc()
    else:
        in_maps = _make_in_maps_general(x, Wk, bk, Wq, bq, Wv, bv, Wp, bp)
        key = ("gen", None, (1, 2, 3))
        if key not in _CACHE:
            _CACHE[key] = _build_general(None, (1, 2, 3))
        nc = _CACHE[key]
    res = run_bass_kernel_spmd(nc, in_maps, core_ids=list(range(N_CORES)), **run_kwargs)
    out = np.empty((B, T, C), dtype=np.float32)
    for b in range(B):
        out[b] = res.results[2 * b]["out"] + res.results[2 * b + 1]["out"]
    return out


# ---------------------------------------------------------------------------
# general path (nonzero biases): original f32r kernel, phases separable
# ---------------------------------------------------------------------------

def _build_general(bench_loops=None, phases=(1, 2, 3)):
    import contextlib

    def _rep(tc):
        return tc.For_i(0, bench_loops, 1) if bench_loops else contextlib.nullcontext()

    nc = bacc.Bacc()
    xT = nc.declare_dram_parameter("xT", [C, T], F32R, isOutput=False)
    wqT = nc.declare_dram_parameter("wqT", [C, MPC], F32R, isOutput=False)
    wkT = nc.declare_dram_parameter("wkT", [C, MPC], F32R, isOutput=False)
    wvT = nc.declare_dram_parameter("wvT", [C, MPC], F32R, isOutput=False)
    wpT = nc.declare_dram_parameter("wpT", [MPC, C], F32R, isOutput=False)
    bq = nc.declare_dram_parameter("bq", [1, MPC], F32R, isOutput=False)
    bk = nc.declare_dram_parameter("bk", [1, MPC], F32R, isOutput=False)
    bv = nc.declare_dram_parameter("bv", [1, MPC], F32R, isOutput=False)
    bp = nc.declare_dram_parameter("bp", [1, C], F32R, isOutput=False)
    maskd = nc.declare_dram_parameter("maskd", [128, 128], F32, isOutput=False)
    mask01d = nc.declare_dram_parameter("mask01", [128, 128], F32R, isOutput=False)
    outp = nc.declare_dram_parameter("out", [T, C], F32, isOutput=True)
    yT_dram = nc.dram_tensor("yT_scratch", [MPC, T], F32R)

    with tile.TileContext(nc) as tc:
        with ExitStack() as ctx:
            persist = ctx.enter_context(tc.tile_pool(name="persist", bufs=1))
            pool_p = ctx.enter_context(tc.tile_pool(name="pool_p", bufs=2))
            ps_big = ctx.enter_context(tc.tile_pool(name="ps_big", bufs=2, space="PSUM"))
            ps_y = ctx.enter_context(tc.tile_pool(name="ps_y", bufs=2, space="PSUM"))

            mask_sb = persist.tile([128, 128], F32, name="mask_sb")
            nc.sync.dma_start(out=mask_sb, in_=maskd[:, :])
            mask01_sb = persist.tile([128, 128], F32R, name="mask01_sb")
            nc.sync.dma_start(out=mask01_sb, in_=mask01d[:, :])
            bias_sb = {}
            for nm, src, wd in (("bq", bq, MPC), ("bk", bk, MPC), ("bv", bv, MPC), ("bp", bp, C)):
                t_ = persist.tile([1, wd], F32R, name=f"{nm}_sb")
                nc.sync.dma_start(out=t_, in_=src[:, :])
                bias_sb[nm] = t_
            ones_f32 = persist.tile([1, 512], F32, name="ones_f32")
            nc.vector.memset(ones_f32, 1.0)
            ones512 = persist.tile([1, 512], F32R, name="ones512")
            nc.vector.tensor_copy(ones512, ones_f32)
            ones_t = persist.tile([1, 128], F32R, name="ones_t")
            nc.vector.tensor_copy(ones_t, ones_f32[:, 0:128])
            ones8_f32 = persist.tile([128, 8], F32, name="ones8_f32")
            nc.vector.memset(ones8_f32, 1.0)

            qT_sb = [persist.tile([128, T], F32R, name=f"qT{m}") for m in range(MT)]
            kT_sb = [persist.tile([128, T], F32R, name=f"kT{m}") for m in range(MT)]
            vAug = [persist.tile([128, HPC * (HD + 1)], F32R, name=f"vAug{t}") for t in range(TT)]

            if 1 not in phases:
                for m in range(MT):
                    nc.sync.dma_start(out=qT_sb[m], in_=xT[0:128, :])
                    nc.sync.dma_start(out=kT_sb[m], in_=xT[128:256, :])
                for t_ in range(TT):
                    nc.sync.dma_start(out=vAug[t_], in_=xT[0:128, 0:HPC * (HD + 1)])
            if 3 in phases and 2 not in phases:
                for m in range(MT):
                    nc.sync.dma_start(out=yT_dram[m * 128:(m + 1) * 128, :], in_=xT[0:128, :])

            with ExitStack() as ctx1:
              if 1 in phases:
                  pool_w = ctx1.enter_context(tc.tile_pool(name="pool_w", bufs=1))
                  pool_xs = ctx1.enter_context(tc.tile_pool(name="pool_xs", bufs=1))
                  wq_t = [pool_w.tile([128, MPC], F32R, name=f"wq{c}") for c in range(CT)]
                  wk_t = [pool_w.tile([128, MPC], F32R, name=f"wk{c}") for c in range(CT)]
                  wv_t = [pool_w.tile([128, MPC], F32R, name=f"wv{c}") for c in range(CT)]
                  for c in range(CT):
                      nc.sync.dma_start(out=wq_t[c], in_=wqT[c * 128:(c + 1) * 128, :])
                      nc.sync.dma_start(out=wk_t[c], in_=wkT[c * 128:(c + 1) * 128, :])
                      nc.sync.dma_start(out=wv_t[c], in_=wvT[c * 128:(c + 1) * 128, :])

                  rep1 = ctx1.enter_context(_rep(tc))
                  for tch in range(TC):
                      t0 = tch * 512
                      xs = []
                      for c in range(CT):
                          x_ = pool_xs.tile([128, 512], F32R, name=f"xs_{tch}_{c}", tag="xs", bufs=9)
                          nc.sync.dma_start(out=x_, in_=xT[c * 128:(c + 1) * 128, t0:t0 + 512])
                          xs.append(x_)
                      for wt, bias, dst in ((wq_t, "bq", qT_sb), (wk_t, "bk", kT_sb)):
                          for m in range(MT):
                              ps = ps_big.tile([128, 512], F32, name=f"ps_{tch}_{bias}_{m}", tag="big", padded_shape=[128, 1024])
                              for c in range(CT):
                                  nc.tensor.matmul(ps, wt[c][:, m * 128:(m + 1) * 128], xs[c],
                                                   start=(c == 0), stop=False)
                              nc.tensor.matmul(ps, bias_sb[bias][:, m * 128:(m + 1) * 128], ones512,
                                               start=False, stop=True)
                              nc.vector.tensor_copy(dst[m][:, t0:t0 + 512], ps)
                      for tt in range(4):
                          tg = tch * 4 + tt
                          ps = ps_big.tile([128, MPC], F32, name=f"psv_{tg}", tag="big", padded_shape=[128, 1024])
                          for c in range(CT):
                              nc.tensor.matmul(ps, xs[c][:, tt * 128:(tt + 1) * 128], wv_t[c],
                                               start=(c == 0), stop=False)
                          nc.tensor.matmul(ps, ones_t, bias_sb["bv"], start=False, stop=True)
                          va = vAug[tg].rearrange("p (h w) -> p h w", w=HD + 1)
                          nc.vector.tensor_copy(va[:, :, 0:HD],
                                                ps.rearrange("p (h w) -> p h w", w=HD))
                          nc.vector.tensor_copy(va[:, :, HD], ones8_f32)

            with ExitStack() as ctx2:
              pool_p2 = ctx2.enter_context(tc.tile_pool(name="pool_p2", bufs=3))
              ctx2.enter_context(_rep(tc) if 2 in phases else nullcontext())
              if 2 in phases:
                  for ic in range(2):
                      i0 = ic * 1024
                      jmax = 8 * ic + 7
                      for h in range(HPC):
                          mt, so = h // 2, (h % 2) * 64
                          kT_h = kT_sb[mt]
                          qT_h = qT_sb[mt]
                          psy = ps_y.tile([HD + 1, 1024], F32, name=f"psy_{h}_{ic}", tag="psy")
                          pend = None

                          def _emit_y(j, P):
                              lo = max(0, j * 128 - i0)
                              for a, b in ((lo, 512), (max(lo, 512), 1024)):
                                  if a < b:
                                      nc.tensor.matmul(psy[:, a:b],
                                                       vAug[j][:, h * (HD + 1):(h + 1) * (HD + 1)],
                                                       P[:, a:b],
                                                       start=(j == 0), stop=(j == jmax))

                          for j in range(jmax + 1):
                              lo = max(0, j * 128 - i0)
                              ps_s = ps_big.tile([128, 1024], F32, name=f"pss_{h}_{ic}_{j}", tag="big")
                              for a, b in ((lo, 512), (max(lo, 512), 1024)):
                                  if a < b:
                                      nc.tensor.matmul(ps_s[:, a:b],
                                                       kT_h[so:so + HD, j * 128:(j + 1) * 128],
                                                       qT_h[so:so + HD, i0 + a:i0 + b],
                                                       start=True, stop=True)
                              P = pool_p.tile([128, 1024], F32R, name=f"P_{h}_{ic}_{j}", tag="P", bufs=3)
                              nc.scalar.activation(out=P[:, lo:1024], in_=ps_s[:, lo:1024],
                                                   func=EXP, scale=SCALE)
                              if j * 128 >= i0:
                                  nc.vector.tensor_mul(P[:, lo:lo + 128], P[:, lo:lo + 128], mask01_sb)
                              if pend is not None:
                                  _emit_y(*pend)
                              pend = (j, P)
                          _emit_y(*pend)
                          yu = pool_p2.tile([HD + 1, 1024], F32, name=f"yu_{h}_{ic}", tag="yu")
                          nc.vector.tensor_copy(yu, psy)
                          r32 = pool_p2.tile([1, 1024], F32, name=f"r32_{h}_{ic}", tag="r32")
                          nc.vector.reciprocal(r32, yu[HD:HD + 1, :])
                          rb = pool_p2.tile([HD, 1024], F32, name=f"rb_{h}_{ic}", tag="rb")
                          nc.gpsimd.partition_broadcast(rb, r32)
                          yst_ = pool_p2.tile([HD, 1024], F32R, name=f"yst_{h}_{ic}", tag="yst")
                          nc.vector.tensor_mul(yst_, yu[0:HD, :], rb)
                          nc.sync.dma_start(out=yT_dram[mt * 128 + so:mt * 128 + so + HD, i0:i0 + 1024],
                                            in_=yst_)

            with ExitStack() as ctx3:
              if 3 in phases:
                  pool_3 = ctx3.enter_context(tc.tile_pool(name="pool_3", bufs=1))
                  wp_t = [pool_3.tile([128, C], F32R, name=f"wp{m}") for m in range(MT)]
                  for m in range(MT):
                      nc.sync.dma_start(out=wp_t[m], in_=wpT[m * 128:(m + 1) * 128, :])
                  rep3 = ctx3.enter_context(_rep(tc))
                  for tt in range(TT):
                      yt = []
                      for m in range(MT):
                          y_ = pool_3.tile([128, 128], F32R, name=f"yt_{tt}_{m}", tag="yt", bufs=6)
                          nc.sync.dma_start(out=y_, in_=yT_dram[m * 128:(m + 1) * 128,
                                                               tt * 128:(tt + 1) * 128])
                          yt.append(y_)
                      for nch in range(2):
                          n0 = nch * 512
                          ps = ps_big.tile([128, 512], F32, name=f"pso_{tt}_{nch}", tag="big", padded_shape=[128, 1024])
                          for m in range(MT):
                              nc.tensor.matmul(ps, yt[m], wp_t[m][:, n0:n0 + 512],
                                               start=(m == 0), stop=False)
                          nc.tensor.matmul(ps, ones_t, bias_sb["bp"][:, n0:n0 + 512],
                                           start=False, stop=True)
                          o_sb = pool_3.tile([128, 512], F32, name=f"o_{tt}_{nch}", tag="o", bufs=3)
                          nc.scalar.copy(o_sb, ps)
                          nc.sync.dma_start(out=outp[tt * 128:(tt + 1) * 128, n0:n0 + 512], in_=o_sb)
    nc.finalize()
    return nc


def _make_in_maps_general(x, Wk, bk, Wq, bq, Wv, bv, Wp, bp):
    x = np.asarray(x, dtype=np.float32)
    Wk, Wq, Wv, Wp = (np.asarray(a, dtype=np.float32) for a in (Wk, Wq, Wv, Wp))
    bk, bq, bv, bp = (np.asarray(a, dtype=np.float32) for a in (bk, bq, bv, bp))

    mask = np.where(np.tril(np.ones((128, 128), dtype=bool)).T, 0.0, -1e9).astype(np.float32)
    xT_b = [np.ascontiguousarray(x[b].T) for b in range(B)]
    in_maps = []
    for c in range(N_CORES):
        b, half = c // 2, c % 2
        hs = half * MPC
        in_maps.append({
            "xT": xT_b[b],
            "wqT": np.ascontiguousarray(Wq[hs:hs + MPC, :].T),
            "wkT": np.ascontiguousarray(Wk[hs:hs + MPC, :].T),
            "wvT": np.ascontiguousarray(Wv[hs:hs + MPC, :].T),
            "wpT": np.ascontiguousarray(Wp[:, hs:hs + MPC].T),
            "bq": bq[hs:hs + MPC].reshape(1, MPC).copy(),
            "bk": bk[hs:hs + MPC].reshape(1, MPC).copy(),
            "bv": bv[hs:hs + MPC].reshape(1, MPC).copy(),
            "bp": (bp if half == 0 else np.zeros_like(bp)).reshape(1, C).copy(),
            "maskd": mask,
            "mask01": np.where(np.tril(np.ones((128, 128), dtype=bool)).T, 1.0, 0.0).astype(np.float32),
        })
    return in_maps
